# revision 73
# baseline (speedup 1.0000x reference)
"""Trainium2 Bass kernel for a pre-norm transformer block (attention + MLP).

Shapes: x [4, 1024, 1024], H=16 heads, Dh=64, MLP hidden 4096, f32.

Strategy (8 NeuronCores, no collectives):
  - Token-sharded: core c handles batch row b=c//2, query tokens
    [off, off+512), off=(c%2)*512. The token axis is ROTATED host-side so
    the query tokens are always columns [0, 512) of the per-core row
    (attention is permutation-invariant over keys), letting the query
    block reuse row-block-0 LN1 stats. Both cores of a pair redundantly
    compute K/V over the full 1024-token row (no cross-core comms).
  - Activations flow transposed [feature(partition), token(free)]; all
    big matmuls are bf16 (213ns/MM at N=512 warm).
  - All three LayerNorms are handled affinely: gains fold into the next
    weight matrix host-side, and  ln(x) @ W'^T = rstd*(x @ W'^T)
    + (-mu*rstd)*rowsum(W')  is applied at PSUM-eviction time, so the
    PE stream never stalls on LN stats (fc1 and fc2 run on raw x2/u).
  - Per-token LN stats via ones-vector matmuls on the PE; rstd/-mu*rstd
    rows are broadcast across partitions with gpsimd.partition_broadcast.
  - Softmax denominator folded into the attention A@V matmul via an
    appended ones-column on V (row 64 of the PSUM output is sum(exp)).
  - The attention tail is exp(ACT)-throughput bound, so half of Q/K and
    all of the second V group are deferred and interleaved into the
    av() cycles as PE filler work.
  - Weights stream in large batched DMAs on the Pool/SWDGE queue
    (bypasses the 625ns/DMA HWDGE issue bottleneck); x and constants on
    the sync queue. fc2 runs in 5 output groups (the last two single)
    with pre-allocated PSUM banks so evictions + output DMAs overlap the
    remaining matmuls, and LN-stat matmuls trail their producers by two
    tiles so the in-order PE queue never waits on an eviction.
"""

import sys

try:
    import concourse  # noqa: F401
except ImportError:  # pragma: no cover
    sys.path.insert(0, "/opt/trn_rl_repo")

import ml_dtypes
import numpy as np

import concourse.bass as bass  # noqa: F401
import concourse.tile as tile
from concourse import bacc, bass_utils, mybir

F32 = mybir.dt.float32
F32R = mybir.dt.float32r
BF16 = mybir.dt.bfloat16
AF = mybir.ActivationFunctionType
OP = mybir.AluOpType

P = 128
C = 1024
N = 1024
B = 4
H = 16
DH = 64
DFF = 4096
TOK = 512          # per-core query-token block
NCT = C // P       # 8 c-tiles
NFT = DFF // P     # 32 d'-tiles
EPS = 1e-5
SCALE = DH ** -0.5

# packed-constant column offsets (cpk)
QS, KS, PB, F1B, F2B, WS1, WS2 = 0, 8, 16, 24, 56, 64, 96
NW = 104

_CACHE = {}


def build():
    nc = bacc.Bacc(
        "TRN2",
        target_bir_lowering=False,
        debug=False,
        enable_asserts=False,
        num_devices=8,
    )

    def din(name, shape, dt=BF16):
        return nc.dram_tensor(name, shape, dt, kind="ExternalInput").ap()

    xrow = din("xrow", [C, N])            # rotated x[b].T (bf16)
    wq = din("wq", [C, C])                # (q rows of qkv_w * ln1_g).T
    wkv = din("wkv", [C, 2 * C])          # (k,v rows, folded).T
    wproj = din("wproj", [C, C])          # proj_w.T
    wfc1 = din("wfc1", [C, DFF])          # (fc1_w * ln2_g).T
    wfc2 = din("wfc2", [DFF, C])          # (fc2_w * lnh_g).T
    cpk = din("cpk", [P, NW], F32)        # packed per-feature constants
    wvs = din("wvs", [1, C], F32R)        # rowsums for V cols (row layout)

    outT = nc.dram_tensor("outT", [C, TOK], F32, kind="ExternalOutput").ap()

    with tile.TileContext(nc) as tc:
        const = tc.alloc_tile_pool(name="const", bufs=1)
        big = tc.alloc_tile_pool(name="big", bufs=1)
        tmp = tc.alloc_tile_pool(name="tmp", bufs=2)
        misc = tc.alloc_tile_pool(name="misc", bufs=1)
        wpool = tc.alloc_tile_pool(name="w", bufs=4)
        w2pool = tc.alloc_tile_pool(name="w2", bufs=4)

        # --- input DMAs first (sync queue) ---
        xr = big.tile([P, NCT, N], BF16, tag="A")      # x[b].T tiled (bf16)
        xrs = xrow.rearrange("(i p) t -> p i t", p=P)
        for ci in range(NCT):
            nc.sync.dma_start(xr[:, ci, :], xrs[:, ci, :])
        gb = const.tile([P, NW], F32, name="cpk")
        nc.sync.dma_start(gb[:], cpk[:])
        wvs_s = const.tile([1, C], F32R)
        nc.sync.dma_start(wvs_s[:], wvs[:])

        # --- weight streams (Pool/SWDGE queue; FIFO order matters) ---
        wqr = wq.rearrange("(i p) c -> p i c", p=P)
        wkvr = wkv.rearrange("(i p) c -> p i c", p=P)
        wpr = wproj.rearrange("(i p) c -> p i c", p=P)
        wf1r = wfc1.rearrange("(i p) c -> p i c", p=P)

        def wload(src, g, nm):
            t = wpool.tile([P, NCT, TOK], BF16, tag="wg", name=nm)
            nc.gpsimd.dma_start(t[:], src[:, :, g * TOK:(g + 1) * TOK])
            return t

        def wload_sync(src_, g, nm):
            t = wpool.tile([P, NCT, TOK], BF16, tag="wg", name=nm)
            return t

        wqg = {0: wload_sync(wqr, 0, "wq0")}
        wkg = {0: wload_sync(wkvr, 0, "wk0")}
        for half in range(2):
            hs = slice(half * 256, (half + 1) * 256)
            nc.sync.dma_start(wqg[0][:, :, hs], wqr[:, :, hs])
            nc.sync.dma_start(wkg[0][:, :, hs], wkvr[:, :, hs])
        wvg = {}
        wpg = {}
        wf1g = {}

        # --- constants via memset (no DMA) ---
        ones1 = const.tile([1, P], F32)
        nc.vector.memset(ones1[:], 1.0)
        ones1h = const.tile([DH + 1, P], F32)   # ones row AT partition 64
        nc.vector.memset(ones1h[DH:DH + 1, :], 1.0)
        ones128b = const.tile([P, 1], BF16)
        nc.vector.memset(ones128b[:], 1.0)
        eps = const.tile([1, 1], F32)
        nc.vector.memset(eps[:], EPS)

        def ln_rows(ps_sum, ps_sq, n_elems, nm):
            """[1,TOK] PSUM sum/sumsq -> (mu f32r, rstd f32r) SBUF rows."""
            inv = 1.0 / n_elems
            mu = misc.tile([1, TOK], F32R, tag="ln_mu", bufs=2, name=f"mu_{nm}")
            nc.vector.tensor_scalar_mul(mu[:], ps_sum[:], inv)
            ex2 = misc.tile([1, TOK], F32, tag="ln_ex2", bufs=2, name=f"ex2_{nm}")
            nc.vector.tensor_scalar(ex2[:], ps_sq[:], inv, EPS, op0=OP.mult, op1=OP.add)
            mu2 = misc.tile([1, TOK], F32, tag="ln_mu2", bufs=2, name=f"mu2_{nm}")
            nc.vector.tensor_mul(mu2[:], mu[:], mu[:])
            nc.vector.tensor_sub(ex2[:], ex2[:], mu2[:])      # var+eps, in place
            rvar = misc.tile([1, TOK], F32, tag="ln_rv", bufs=2, name=f"rv_{nm}")
            nc.vector.reciprocal(rvar[:], ex2[:])
            rstd = misc.tile([1, TOK], F32R, tag="ln_rstd", bufs=2, name=f"rstd_{nm}")
            nc.scalar.activation(rstd[:], rvar[:], AF.Sqrt)
            return mu, rstd

        def ln_bcast(mu, rstd, nm):
            """rows -> [P,TOK] bf16 broadcast tiles (rstd_bc, nmr_bc)."""
            r16 = misc.tile([1, TOK], BF16, tag="ln_r16", bufs=2, name=f"r16_{nm}")
            nc.vector.tensor_copy(r16[:], rstd[:])
            n16 = misc.tile([1, TOK], BF16, tag="ln_n16", bufs=2, name=f"n16_{nm}")
            nc.vector.scalar_tensor_tensor(n16[:], mu[:], -1.0, rstd[:],
                                           op0=OP.mult, op1=OP.mult)
            r_bc = misc.tile([P, TOK], BF16, tag="lnbc", bufs=4, name=f"rbc_{nm}")
            nc.gpsimd.partition_broadcast(r_bc[:], r16[:])
            n_bc = misc.tile([P, TOK], BF16, tag="lnbc", bufs=4, name=f"nbc_{nm}")
            nc.gpsimd.partition_broadcast(n_bc[:], n16[:])
            return r_bc, n_bc

        # --- LN1 stats for the two row blocks (blk 0 = query block) ---
        ps_stat = tc.alloc_tile_pool(name="ps_stat1", bufs=2, space="PSUM")
        ps_tp = tc.alloc_tile_pool(name="ps_tp", bufs=2, space="PSUM")
        ps_warm = tc.alloc_tile_pool(name="ps_warm", bufs=1, space="PSUM")
        warm_rhs = const.tile([P, TOK], BF16, name="warm_rhs")
        nc.vector.memset(warm_rhs[:], 0.0)
        warm_ps = ps_warm.tile([1, TOK], F32, tag="warm")

        def warm(n):
            for _ in range(n):
                nc.tensor.matmul(warm_ps[:], ones128b[:], warm_rhs[:],
                                 start=True, stop=True)
        rstd_sb = []
        nmr_sb = []
        rstdT = misc.tile([P, NCT], F32, tag="rstdT")   # column form per tok-tile
        nmrT = misc.tile([P, NCT], F32, tag="nmrT")
        for blk in range(2):
            sl = slice(blk * TOK, (blk + 1) * TOK)
            ps_su = ps_stat.tile([1, TOK], F32, tag="ln_s")
            ps_sq = ps_stat.tile([1, TOK], F32, tag="ln_q")
            for ci in range(NCT):
                s = xr[:, ci, sl]
                sq = tmp.tile([P, TOK], BF16, tag="ln_sq")
                nc.scalar.activation(sq[:], s, AF.Square)
                nc.tensor.matmul(ps_su[:], ones128b[:], s,
                                 start=(ci == 0), stop=(ci == NCT - 1))
                nc.tensor.matmul(ps_sq[:], ones128b[:], sq[:],
                                 start=(ci == 0), stop=(ci == NCT - 1))
                if blk == 0:
                    warm(1)
            mu, rstd = ln_rows(ps_su, ps_sq, C, f"l1b{blk}")
            nmr = misc.tile([1, TOK], F32R, tag="ln_nmr", bufs=2, name=f"nmr{blk}")
            nc.vector.scalar_tensor_tensor(nmr[:], mu[:], -1.0, rstd[:],
                                           op0=OP.mult, op1=OP.mult)
            r_bc, n_bc = ln_bcast(mu, rstd, f"l1b{blk}")
            rstd_sb.append(r_bc)
            nmr_sb.append(n_bc)
            # transpose rstd/nmr rows into per-token-tile columns (for V)
            for sub in range(4):
                r = blk * 4 + sub
                cs = slice(sub * P, (sub + 1) * P)
                pt = ps_tp.tile([P, 1], F32, tag="tp", name=f"tp{r}")
                nc.tensor.matmul(pt[:], rstd[0:1, cs].bitcast(F32),
                                 ones1[0:1, 0:1],
                                 start=True, stop=True)
                nc.vector.tensor_copy(rstdT[:, r:r + 1], pt[:])
                pt2 = ps_tp.tile([P, 1], F32, tag="tp", name=f"tp2_{r}")
                nc.tensor.matmul(pt2[:], nmr[0:1, cs].bitcast(F32),
                                 ones1[0:1, 0:1],
                                 start=True, stop=True)
                nc.vector.tensor_copy(nmrT[:, r:r + 1], pt2[:])
        # broadcast V-column rowsums to all partitions (once)
        wvs16 = misc.tile([1, C], BF16, tag="wvs16")
        nc.vector.tensor_copy(wvs16[:], wvs_s[:])
        wvs_b = misc.tile([P, C], BF16, tag="wvs_b")
        nc.gpsimd.partition_broadcast(wvs_b[:], wvs16[:])
        ps_warm.release()
        ps_tp.release()
        ps_stat.release()

        # deferred weight streams queue behind the LN1 broadcasts
        # ring slots chosen so these SWDGE gens WAR-wait on the wq0/wk0
        # buffers (~20-28us) instead of polluting the startup DMA stream
        wvg[0] = wpool.tile([P, NCT, TOK], BF16, tag="wg", name="wv0")
        wvg[1] = wpool.tile([P, NCT, TOK], BF16, tag="wg", name="wv1")
        wqg[1] = wpool.tile([P, NCT, TOK], BF16, tag="wg", name="wq1")
        wkg[1] = wpool.tile([P, NCT, TOK], BF16, tag="wg", name="wk1")
        nc.gpsimd.dma_start(wqg[1][:], wqr[:, :, TOK:2 * TOK])
        nc.gpsimd.dma_start(wkg[1][:], wkvr[:, :, TOK:2 * TOK])
        nc.gpsimd.dma_start(wvg[0][:], wkvr[:, :, 2 * TOK:3 * TOK])
        nc.gpsimd.dma_start(wvg[1][:], wkvr[:, :, 3 * TOK:4 * TOK])
        wpg[0] = wpool.tile([P, NCT, TOK], BF16, tag="wg", name="wp0")
        nc.sync.dma_start(wpg[0][:], wpr[:, :, 0:TOK])

        # --- QKV with fused LN1 (transposed QT/KT, natural V + ones col) ---
        KT = big.tile([P, NCT, N], BF16, tag="B")
        QT = big.tile([P, NCT, TOK], BF16, tag="F")
        V = big.tile([P, NCT, H, DH + 1], BF16, tag="V")
        for r in range(NCT):
            nc.vector.memset(V[:, r, :, DH:DH + 1], 1.0)

        ps_accA = tc.alloc_tile_pool(name="ps_qkvA", bufs=4, space="PSUM")
        accpool = {"cur": ps_accA}

        def q_chunk(g, jt):
            jj = g * 4 + jt
            ps = accpool["cur"].tile([P, TOK], F32, tag="acc")
            for ci in range(NCT):
                nc.tensor.matmul(ps[:], wqg[g][:, ci, jt * P:(jt + 1) * P],
                                 xr[:, ci, 0:TOK],
                                 start=(ci == 0), stop=(ci == NCT - 1))
            t = tmp.tile([P, TOK], BF16, tag="ev", bufs=3)
            nc.vector.tensor_mul(t[:], ps[:], rstd_sb[0][:])
            nc.vector.scalar_tensor_tensor(
                QT[:, jj, :], nmr_sb[0][:], gb[:, QS + jj:QS + jj + 1], t[:],
                op0=OP.mult, op1=OP.add)

        def k_chunk(g, jt):
            jj = g * 4 + jt
            for blk in range(2):
                ps = accpool["cur"].tile([P, TOK], F32, tag="acc")
                for ci in range(NCT):
                    nc.tensor.matmul(ps[:], wkg[g][:, ci, jt * P:(jt + 1) * P],
                                     xr[:, ci, blk * TOK:(blk + 1) * TOK],
                                     start=(ci == 0), stop=(ci == NCT - 1))
                t = tmp.tile([P, TOK], BF16, tag="ev", bufs=3)
                nc.vector.tensor_mul(t[:], ps[:], rstd_sb[blk][:])
                nc.vector.scalar_tensor_tensor(
                    KT[:, jj, blk * TOK:(blk + 1) * TOK],
                    nmr_sb[blk][:], gb[:, KS + jj:KS + jj + 1], t[:],
                    op0=OP.mult, op1=OP.add)

        def v_chunk(g, rs):
            h0 = 8 * g
            dsl = slice(g * TOK, (g + 1) * TOK)
            for r in rs:
                ps = accpool["cur"].tile([P, TOK], F32, tag="acc")
                for ci in range(NCT):
                    nc.tensor.matmul(ps[:], xr[:, ci, r * P:(r + 1) * P],
                                     wvg[g][:, ci, :],
                                     start=(ci == 0), stop=(ci == NCT - 1))
                t = tmp.tile([P, TOK], BF16, tag="ev", bufs=3)
                nc.vector.tensor_scalar_mul(t[:], wvs_b[:, dsl], nmrT[:, r:r + 1])
                nc.vector.scalar_tensor_tensor(
                    V[:, r, h0:h0 + 8, 0:DH],
                    ps[:].rearrange("p (h d) -> p h d", h=8),
                    rstdT[:, r:r + 1],
                    t[:].rearrange("p (h d) -> p h d", h=8),
                    op0=OP.mult, op1=OP.add)

        Es = {}

        def s_exp(jj):
            E_l = []
            for kt in range(NCT):
                E_t = big.tile([P, 2 * TOK], BF16, tag="E", bufs=12,
                               name=f"E{jj}_{kt}")
                E_l.append(E_t)
                ks = slice(kt * P, (kt + 1) * P)
                ps2 = ps_s.tile([P, 2, TOK], F32, tag="S", name=f"S{jj}_{kt}")
                nc.tensor.matmul(ps2[:, 0, :], KT[0:64, jj, ks], QT[0:64, jj, :],
                                 start=True, stop=True, tile_position=(0, 0))
                nc.tensor.matmul(ps2[:, 1, :], KT[64:128, jj, ks], QT[64:128, jj, :],
                                 start=True, stop=True, tile_position=(64, 0))
                nc.scalar.activation(
                    E_t[:].rearrange("p (a t) -> p a t", a=2), ps2[:],
                    AF.Exp, scale=SCALE)
            Es[jj] = E_l

        def av(jj):
            E_l = Es.pop(jj)
            for half in range(2):
                h = 2 * jj + half
                es = slice(half * TOK, (half + 1) * TOK)
                po = ps_o2.tile([P, TOK], F32, tag="O")
                for kt in range(NCT):
                    nc.tensor.matmul(po[0:DH + 1, :], V[:, kt, h, :], E_l[kt][:, es],
                                     start=(kt == 0), stop=(kt == NCT - 1))
                rec = misc.tile([DH + 1, TOK], F32R, tag="rec", bufs=2)
                with nc.allow_low_precision(reason="softmax denom to f32r bcast"):
                    nc.vector.reciprocal(rec[DH:DH + 1, :], po[DH:DH + 1, :])
                pl2 = ps_s.tile([P, 2, TOK], F32, tag="S", name=f"lbc{jj}_{half}")
                pl = pl2[:, 0, :]
                nc.tensor.matmul(pl2[0:DH, 0, :], ones1h[DH:DH + 1, 0:DH].bitcast(F32R),
                                 rec[DH:DH + 1, :], start=True, stop=True)
                pls = misc.tile([DH, TOK], BF16, tag="pls", bufs=2)
                nc.vector.tensor_copy(pls[:], pl[0:DH, :])
                if half == 0:
                    nc.vector.tensor_mul(OT[0:DH, jj, :], po[0:DH, :], pls[:])
                else:
                    sh = misc.tile([DH, TOK], BF16, tag="shift", bufs=2)
                    nc.vector.tensor_mul(sh[:], po[0:DH, :], pls[:])
                    nc.gpsimd.dma_start(OT[DH:P, jj, :], sh[:])

        OT = big.tile([P, NCT, TOK], BF16, tag="C")

        for jt in range(4):
            q_chunk(0, jt)
        for jt in range(4):
            k_chunk(0, jt)
        ps_accA.release()
        ps_s = tc.alloc_tile_pool(name="ps_s", bufs=2, space="PSUM")
        ps_o2 = tc.alloc_tile_pool(name="ps_o2", bufs=2, space="PSUM")
        ps_acc = tc.alloc_tile_pool(name="ps_qkvB", bufs=2, space="PSUM")
        accpool["cur"] = ps_acc
        s_exp(0)
        s_exp(1)
        q_chunk(1, 0)
        k_chunk(1, 0)
        v_chunk(0, range(4))
        v_chunk(0, range(4, 8))
        # deferred PE work interleaved into the ACT(exp)-paced av cycles
        fill = {
            0: [(q_chunk, (1, 1)), (k_chunk, (1, 1))],
            1: [(q_chunk, (1, 2)), (k_chunk, (1, 2))],
            2: [(v_chunk, (1, range(4)))],
            3: [(v_chunk, (1, range(4, 8)))],
            4: [(q_chunk, (1, 3))],
            5: [(k_chunk, (1, 3))],
        }
        x2 = big.tile([P, NCT, TOK], BF16, tag="B")   # reuses KT slot
        prog = {}

        def wp_slice(ig, ci, cs):
            return wpg[ig][:, ci, cs]

        def proj_evict(i):
            nc.vector.scalar_tensor_tensor(
                x2[:, i, :], prog.pop(i)[:], gb[:, PB + i:PB + i + 1],
                xr[:, i, 0:TOK], op0=OP.add, op1=OP.add)

        for jj in range(NCT):
            av(jj)
            for fn, args in fill.get(jj, ()):
                fn(*args)
            if jj + 2 < NCT:
                s_exp(jj + 2)
        for p_ in (ps_acc, ps_o2, ps_s):
            p_.release()

        # late weight streams: queued after the av OT-shifts on purpose
        wpg[1] = wload(wpr, 1, "wp1")
        wf1g[0] = wload(wf1r, 0, "wf1_0")
        wf1g[1] = wload(wf1r, 1, "wf1_1")


        # --- remaining projection chains -> x2 (bf16; LN2 stats inline) ---
        ps_stat = tc.alloc_tile_pool(name="ps_stat2", bufs=1, space="PSUM")
        st_s = ps_stat.tile([1, TOK], F32, tag="ln_s")
        st_q = ps_stat.tile([1, TOK], F32, tag="ln_q")
        ps_acc = tc.alloc_tile_pool(name="ps_proj", bufs=4, space="PSUM")
        def proj_stats(i):
            sq = tmp.tile([P, TOK], BF16, tag="ln_sq")
            nc.scalar.activation(sq[:], x2[:, i, :], AF.Square)
            nc.tensor.matmul(st_s[:], ones128b[:], x2[:, i, :],
                             start=(i == 0), stop=(i == NCT - 1))
            nc.tensor.matmul(st_q[:], ones128b[:], sq[:],
                             start=(i == 0), stop=(i == NCT - 1))

        for i in range(NCT):
            prog[i] = ps_acc.tile([P, TOK], F32, tag="acc", name=f"prj{i}")
            for ci in range(NCT):
                nc.tensor.matmul(prog[i][:],
                                 wp_slice(i // 4, ci, slice((i % 4) * P, (i % 4 + 1) * P)),
                                 OT[:, ci, :], start=(ci == 0), stop=(ci == NCT - 1))
            if i >= 2:
                proj_stats(i - 2)
            proj_evict(i)
        proj_stats(NCT - 2)
        proj_stats(NCT - 1)
        ps_acc.release()

        # --- LN2 rows + broadcast (fc1 matmuls do NOT wait on these) ---
        mu, rstd = ln_rows(st_s, st_q, C, "l2")
        rstd2_bc, nmr2_bc = ln_bcast(mu, rstd, "l2")
        ps_stat.release()

        # --- fc1 on raw x2 + fused LN2 + gelu -> U ---
        U0 = big.tile([P, NFT // 2, TOK], BF16, tag="V")   # reuses V slot
        U1 = big.tile([P, NFT // 2, TOK], BF16, tag="A")   # reuses xr slot

        def u_tile(i):
            return (U0 if i < NFT // 2 else U1)[:, i % (NFT // 2), :]

        # fc2 PSUM banks reserved now so its chains never WAR-wait on fc1
        ps_fc2 = tc.alloc_tile_pool(name="ps_fc2", bufs=1, space="PSUM")
        ps_stath = tc.alloc_tile_pool(name="ps_stath", bufs=1, space="PSUM")
        sh_s = ps_stath.tile([1, TOK], F32, tag="lnh_s")
        sh_q = ps_stath.tile([1, TOK], F32, tag="lnh_q")
        ps_acc = tc.alloc_tile_pool(name="ps_fc1", bufs=3, space="PSUM")

        def fc1_stats(i):
            sq = tmp.tile([P, TOK], BF16, tag="ln_sq")
            nc.scalar.activation(sq[:], u_tile(i), AF.Square)
            nc.tensor.matmul(sh_s[:], ones128b[:], u_tile(i),
                             start=(i == 0), stop=(i == NFT - 1))
            nc.tensor.matmul(sh_q[:], ones128b[:], sq[:],
                             start=(i == 0), stop=(i == NFT - 1))

        for ig in range(8):
            wt = wf1g.pop(ig)
            if ig + 2 < 8:
                wf1g[ig + 2] = wload(wf1r, ig + 2, f"wf1_{ig + 2}")
            for i4 in range(4):
                i = ig * 4 + i4
                ps = ps_acc.tile([P, TOK], F32, tag="acc")
                for ci in range(NCT):
                    nc.tensor.matmul(ps[:], wt[:, ci, i4 * P:(i4 + 1) * P],
                                     x2[:, ci, :], start=(ci == 0), stop=(ci == NCT - 1))
                if i >= 2:
                    fc1_stats(i - 2)
                t1 = tmp.tile([P, TOK], BF16, tag="ev", bufs=3)
                nc.vector.tensor_mul(t1[:], ps[:], rstd2_bc[:])
                t2 = tmp.tile([P, TOK], BF16, tag="ev2", bufs=3)
                nc.vector.scalar_tensor_tensor(
                    t2[:], nmr2_bc[:], gb[:, WS1 + i:WS1 + i + 1], t1[:],
                    op0=OP.mult, op1=OP.add)
                nc.scalar.activation(u_tile(i), t2[:], AF.Gelu,
                                     bias=gb[:, F1B + i:F1B + i + 1])
        fc1_stats(NFT - 2)
        fc1_stats(NFT - 1)
        ps_acc.release()

        # --- fc2 weight stream starts ahead of the LNh broadcast on the
        # Pool queue; fc2 matmuls run on raw u (LNh fused at eviction) ---
        wf2r = wfc2.rearrange("(i p) c -> p i c", p=P)

        G_J0 = [0, 2, 4, 6, 7]
        G_NJ = [2, 2, 2, 1, 1]

        def w2load(g, c4):
            nj = G_NJ[g]
            ni = 8 if nj == 2 else 16          # same bytes per chunk either way
            t = w2pool.tile([P, 16, P], BF16, tag="w2", name=f"w2_{g}_{c4}")
            tv = t[:].rearrange("p (a b) c -> p a (b c)", b=16 // ni)
            nc.gpsimd.dma_start(
                tv[:, 0:ni, 0:nj * P],
                wf2r[:, c4 * ni:(c4 + 1) * ni, G_J0[g] * P:G_J0[g] * P + nj * P])
            return t

        CHUNKS = [(g_, c_) for g_ in range(5) for c_ in range(4 if G_NJ[g_] == 2 else 2)]
        CIDX = {ch: k for k, ch in enumerate(CHUNKS)}
        w2tiles = {ch: w2load(*ch) for ch in CHUNKS[:2]}

        mu, rstd = ln_rows(sh_s, sh_q, DFF, "lh")
        rstdh_bc, nmrh_bc = ln_bcast(mu, rstd, "lh")
        ps_stath.release()

        CB = big.tile([P, 4, TOK], F32, tag="F")   # reuses QT slot
        GROUPS = [(0, 1), (2, 3), (4, 5), (6,), (7,)]
        for g, js in enumerate(GROUPS):
            corr = []
            for jo, j in enumerate(js):
                nc.vector.scalar_tensor_tensor(
                    CB[:, j % 4, :], nmrh_bc[:], gb[:, WS2 + j:WS2 + j + 1],
                    x2[:, j, :], op0=OP.mult, op1=OP.add)
                corr.append(CB[:, j % 4, :])
            fps = [ps_fc2.tile([P, TOK], F32, tag=f"f2{'ab'[jo]}",
                               bufs=2 if jo == 0 else 1, name=f"fp{j}")
                   for jo, j in enumerate(js)]
            nch = 4 if len(js) == 2 else 2
            nper = NFT // nch
            for c4 in range(nch):
                wt = w2tiles.pop((g, c4))
                k = CIDX[(g, c4)]
                if k + 2 < len(CHUNKS):
                    w2tiles[CHUNKS[k + 2]] = w2load(*CHUNKS[k + 2])
                wv_ = wt[:].rearrange("p a c -> p (a c)").rearrange(
                    "p (a c) -> p a c", c=len(js) * P)
                for ii in range(nper):
                    i = c4 * nper + ii
                    for jo, j in enumerate(js):
                        nc.tensor.matmul(fps[jo][:],
                                         wv_[:, ii, jo * P:(jo + 1) * P], u_tile(i),
                                         start=(i == 0), stop=(i == NFT - 1))
            last = (g == len(GROUPS) - 1)
            for jo, j in enumerate(js):
                t = tmp.tile([P, TOK], F32, tag="fev", bufs=2)
                ot = tmp.tile([P, TOK], F32, tag="out", bufs=2)
                for hs in ([slice(0, 256), slice(256, TOK)] if last else [slice(0, TOK)]):
                    nc.vector.tensor_mul(t[:, hs], fps[jo][:, hs], rstdh_bc[:, hs])
                    nc.vector.scalar_tensor_tensor(
                        ot[:, hs], t[:, hs], gb[:, F2B + j:F2B + j + 1],
                        corr[jo][:, hs] if last else corr[jo],
                        op0=OP.add, op1=OP.add)
                    nc.sync.dma_start(outT[j * P:(j + 1) * P, hs], ot[:, hs])
        ps_fc2.release()

        for p_ in (w2pool, wpool, misc, tmp, big, const):
            p_.release()

    nc.compile()
    return nc


def _prep_inputs(inputs):
    """Host-side transposes/folds/rotations -> per-core in_maps."""
    f = lambda a: np.asarray(a, dtype=np.float32)
    x = f(inputs["x"])
    xT = np.ascontiguousarray(x.transpose(0, 2, 1))          # [B, C, N]

    g1, b1 = f(inputs["ln1_g"]), f(inputs["ln1_b"])
    g2, b2 = f(inputs["ln2_g"]), f(inputs["ln2_b"])
    ghv, bhv = f(inputs["lnh_g"]), f(inputs["lnh_b"])
    for nm, bb in (("ln1_b", b1), ("ln2_b", b2), ("lnh_b", bhv)):
        if np.abs(bb).max() != 0.0:
            raise NotImplementedError(f"{nm} != 0 not supported by this kernel")

    qkv_f = f(inputs["qkv_w"]) * g1[None, :]      # fold ln1_g
    fc1_f = f(inputs["fc1_w"]) * g2[None, :]      # fold ln2_g
    fc2_f = f(inputs["fc2_w"]) * ghv[None, :]     # fold lnh_g
    qs = qkv_f.sum(axis=1)                        # [3072] rowsums

    cpk = np.zeros((P, NW), np.float32)
    cpk[:, QS:QS + 8] = qs[:C].reshape(NCT, P).T
    cpk[:, KS:KS + 8] = qs[C:2 * C].reshape(NCT, P).T
    cpk[:, PB:PB + 8] = f(inputs["proj_b"]).reshape(NCT, P).T
    cpk[:, F1B:F1B + 32] = f(inputs["fc1_b"]).reshape(NFT, P).T
    cpk[:, F2B:F2B + 8] = f(inputs["fc2_b"]).reshape(NCT, P).T
    cpk[:, WS1:WS1 + 32] = fc1_f.sum(axis=1).reshape(NFT, P).T
    cpk[:, WS2:WS2 + 8] = fc2_f.sum(axis=1).reshape(NCT, P).T

    bf = ml_dtypes.bfloat16
    common = {
        "wq": np.ascontiguousarray(qkv_f[:C].T.astype(bf)),
        "wkv": np.ascontiguousarray(qkv_f[C:].T.astype(bf)),
        "wproj": np.ascontiguousarray(f(inputs["proj_w"]).T.astype(bf)),
        "wfc1": np.ascontiguousarray(fc1_f.T.astype(bf)),
        "wfc2": np.ascontiguousarray(fc2_f.T.astype(bf)),
        "cpk": cpk,
        "wvs": np.ascontiguousarray(qs[2 * C:].reshape(1, C)),
    }
    in_maps = []
    for c in range(8):
        b, off = c // 2, (c % 2) * TOK
        m = dict(common)
        m["xrow"] = np.ascontiguousarray(
            np.roll(xT[b], -off, axis=1).astype(bf))
        in_maps.append(m)
    return in_maps


def _assemble(results):
    out = np.empty((B, N, C), np.float32)
    for c in range(8):
        b, off = c // 2, (c % 2) * TOK
        out[b, off:off + TOK, :] = results[c]["outT"].T
    return out


def kernel(**inputs) -> np.ndarray:
    nc = _CACHE.get("nc")
    if nc is None:
        nc = build()
        _CACHE["nc"] = nc
    in_maps = _prep_inputs(inputs)
    res = bass_utils.run_bass_kernel_spmd(nc, in_maps, core_ids=list(range(8)))
    return _assemble(res.results)
